# revision 50
# baseline (speedup 1.0000x reference)
"""Multi-head attention (B=2, S=2048, d_model=768, H=12) on 8 TRN2 NeuronCores.

Sharding: 2-way data parallel over batch x 4-way tensor parallel over heads
(3 heads / 192-wide d_model slice per core). Host compacts masked keys away
(gather of unmasked key/value rows), pads to a 128 multiple, and passes a 0/1
validity vector; softmax needs no mask handling on device (pad keys get V=0
and a 0 in the denominator ones-column). Per core:

    Q^T [192,2048], K^T [192,KP] via projections (dq on partitions)
    V   [KP,192] natural layout, x3 per-head [V_h | valid] blocks
    per head: scores^T[k,q] = K_h^T.T @ Q_h^T ; es = exp(s/8) on ACT
              ctx'^T[65,q] += [V_h|valid].T @ es  (row 64 = denominator)
              ctx = ctx * recip(denom) (DVE + gpsimd partition_broadcast)
    out_partial[2048,768] = ctx^T.T @ Wo_g (bf16), summed on host + bo.

v2 layout decisions (from baseline trace analysis):
  - ACT runs exp only during attention; projection bias-evicts happen on ACT
    while it is otherwise idle (before the first exp); norm + O-evict on DVE.
  - PSUM: sp tag 2x[128,1024] (scores dbuf + proj g0), ctx tag 1x[128,1536]
    (3 per-head banks + proj g1), opo tag 1x[128,384] (O-proj + V-proj).
  - heads 0/1 scores pair in PE row groups 0:64/64:128 (concurrent matmuls);
    head 2 pairs key-tiles t/t+1 the same way via partition-duplicated
    KT1D/QT1D (dup done by SBUF->SBUF DMA).
  - DMA order: weights, xk, xq half 0, xv, xq half 1 -> compute starts ~2us.
  - out partials are bf16 (halves exit traffic), summed f32 on host.
"""

import math

import numpy as np

B = 2
S = 2048
DM = 768
H = 12
DH = 64
G = 4              # head-group (tensor-parallel) degree
HPG = H // G       # heads per core
DQ = HPG * DH      # 192 d_model slice per core
NCORES = 8
P = 128
NKT = DM // P      # 6 contraction tiles for projections

_prog_cache = {}


def _chunks(total, step):
    out = []
    o = 0
    while o < total:
        w = min(step, total - o)
        out.append((o, w))
        o += w
    return out


def _build_nc(KP):
    import concourse.bass as bass
    import concourse.mybir as mybir
    import concourse.tile as tile
    from concourse import bacc

    F32 = mybir.dt.float32
    BF = mybir.dt.bfloat16
    AFT = mybir.ActivationFunctionType

    T = KP // P
    KCH = _chunks(KP, 1024)
    QPCH = _chunks(S, 1024)      # projection chunks for Q
    ACH = _chunks(S, 512)        # attention query chunks
    NCH = _chunks(DM, 384)       # O-proj output chunks (1 PSUM bank each)

    nc = bacc.Bacc(None, target_bir_lowering=False)
    xqT = nc.declare_dram_parameter("xqT", [DM, S], BF, isOutput=False)
    xkT = nc.declare_dram_parameter("xkT", [DM, KP], BF, isOutput=False)
    xvT = nc.declare_dram_parameter("xvT", [DM, KP], BF, isOutput=False)
    # host pre-arranges projection weights to [128, ...] so the upload is
    # one contiguous stripe per partition (the old "(kt p) m -> p kt m"
    # rearrange fragmented into 768 tiny descriptors)
    wq = nc.declare_dram_parameter("wq", [P, NKT * DQ], BF, isOutput=False)
    wk = nc.declare_dram_parameter("wk", [P, NKT * DQ], BF, isOutput=False)
    wv = nc.declare_dram_parameter("wv", [P, NKT * DQ], BF, isOutput=False)
    wo = nc.declare_dram_parameter("wo", [DQ, DM], BF, isOutput=False)
    bq = nc.declare_dram_parameter("bq", [DQ, 1], F32, isOutput=False)
    bk = nc.declare_dram_parameter("bk", [DQ, 1], F32, isOutput=False)
    bv = nc.declare_dram_parameter("bv", [1, DQ], F32, isOutput=False)
    vm = nc.declare_dram_parameter("vm", [P, T], F32, isOutput=False)
    out = nc.declare_dram_parameter("out", [S, DM], BF, isOutput=True)

    with tile.TileContext(nc) as tc:
        with (
            tc.tile_pool(name="persist", bufs=1) as persist,
            tc.tile_pool(name="acts", bufs=6) as acts,
            tc.tile_pool(name="es", bufs=6) as espool,
            tc.tile_pool(name="norm", bufs=4) as norm,
            tc.tile_pool(name="osb", bufs=6) as osb,
            tc.tile_pool(name="ctxs", bufs=2) as ctxs,
            tc.tile_pool(name="ps", bufs=1, space="PSUM") as ps,
        ):
            # ---- DMAs in startup-critical order: the sync engine generates
            # descriptors serially (~0.7us each), so K-proj inputs go first
            WK = persist.tile([P, NKT, DQ], BF, tag="WK")
            WQ = persist.tile([P, NKT, DQ], BF, tag="WQ")
            WV = persist.tile([P, NKT, DQ], BF, tag="WV")
            BK0 = persist.tile([P, 1], F32, tag="BK0")
            BK1 = persist.tile([DH, 1], F32, tag="BK1")
            nc.sync.dma_start(out=WK, in_=wk[:, :].rearrange("p (kt m) -> p kt m", m=DQ))
            nc.sync.dma_start(out=BK0, in_=bk[0:P, :])
            nc.sync.dma_start(out=BK1, in_=bk[P:DQ, :])
            XH = _chunks(KP, (KP + 255) // 256 * 128)
            XK = []
            for kt in range(NKT):
                xt = acts.tile([P, KP], BF, tag="xk", name=f"xk{kt}")
                for (h0, hw) in XH:
                    nc.sync.dma_start(
                        out=xt[:, h0:h0 + hw],
                        in_=xkT[kt * P:(kt + 1) * P, h0:h0 + hw],
                    )
                XK.append(xt)
            BQ0 = persist.tile([P, 1], F32, tag="BQ0")
            BQ1 = persist.tile([DH, 1], F32, tag="BQ1")
            nc.sync.dma_start(out=WQ, in_=wq[:, :].rearrange("p (kt m) -> p kt m", m=DQ))
            nc.sync.dma_start(out=BQ0, in_=bq[0:P, :])
            nc.sync.dma_start(out=BQ1, in_=bq[P:DQ, :])
            XQ = []
            for kt in range(NKT):
                xt = acts.tile([P, S], BF, tag="xq", name=f"xq{kt}")
                nc.sync.dma_start(out=xt[:, 0:1024], in_=xqT[kt * P:(kt + 1) * P, 0:1024])
                XQ.append(xt)
            nc.sync.dma_start(out=WV, in_=wv[:, :].rearrange("p (kt m) -> p kt m", m=DQ))
            BV = persist.tile([P, DQ], F32, tag="BV")
            nc.sync.dma_start(out=BV, in_=bv[:, :].to_broadcast([P, DQ]))
            VM = persist.tile([P, T], F32, tag="VM")
            nc.sync.dma_start(out=VM, in_=vm[:, :])
            XV = []
            for kt in range(NKT):
                xt = acts.tile([P, KP], BF, tag="xv", name=f"xv{kt}")
                for (h0, hw) in XH:
                    nc.sync.dma_start(
                        out=xt[:, h0:h0 + hw],
                        in_=xvT[kt * P:(kt + 1) * P, h0:h0 + hw],
                    )
                XV.append(xt)
            WO0 = persist.tile([P, DM], BF, tag="WO0")   # wo rows 0:128 (h0,h1)
            WO2 = persist.tile([DH, DM], BF, tag="WO2")  # wo rows 128:192 (h2)
            nc.sync.dma_start(out=WO0, in_=wo[0:P, :])
            nc.sync.dma_start(out=WO2, in_=wo[P:DQ, :])
            for kt in range(NKT):
                nc.sync.dma_start(
                    out=XQ[kt][:, 1024:S], in_=xqT[kt * P:(kt + 1) * P, 1024:S]
                )

            # ---- persistent activations ----
            KT0 = persist.tile([P, KP], BF, tag="KT0")    # heads 0,1 (dq 0:128)
            KT1D = persist.tile([P, KP], BF, tag="KT1D")  # head 2, duplicated rows
            QT0 = persist.tile([P, S], BF, tag="QT0")
            QT1D = persist.tile([P, S], BF, tag="QT1D")
            # ---- PE warm-up: dependency-free matmuls run in the DMA-wait
            # shadow; >3.4us of continuous PE activity trips the HAM clock
            # gate to 8/8 before K-proj starts ----
            WUP = persist.tile([P, P], BF, tag="WUP")
            nc.vector.memset(WUP, 0.0)
            wps = ps.tile([P, P], F32, tag="opo", bufs=1, name="warmup_ps")
            for _w in range(40):
                nc.tensor.matmul(
                    wps, lhsT=WUP, rhs=WUP, start=True, stop=True,
                )

            # V blocks padded to 128 cols (cols 0:64 V, 64 ones, 65:128 zero)
            VP = persist.tile([P, T, HPG * P], BF, tag="VP")
            nc.vector.memset(VP, 0.0)

            # ---- K projection ----
            for (c0, cw) in KCH:
                ps0 = ps.tile([P, 1024], F32, tag="sp", bufs=2, name=f"kps0_{c0}")
                ps1 = ps.tile([DH, 1024], F32, tag="ctx01", bufs=1, name=f"kps1_{c0}")
                for kt in range(NKT):
                    for (h0, hw) in _chunks(cw, 512):
                        nc.tensor.matmul(
                            ps0[:, h0:h0 + hw],
                            lhsT=WK[:, kt, 0:P],
                            rhs=XK[kt][:, c0 + h0:c0 + h0 + hw],
                            start=(kt == 0), stop=(kt == NKT - 1),
                        )
                for kt in range(NKT):
                    for (h0, hw) in _chunks(cw, 512):
                        nc.tensor.matmul(
                            ps1[:, h0:h0 + hw],
                            lhsT=WK[:, kt, P:DQ],
                            rhs=XK[kt][:, c0 + h0:c0 + h0 + hw],
                            start=(kt == 0), stop=(kt == NKT - 1),
                        )
                nc.vector.tensor_scalar_add(
                    KT0[:, c0:c0 + cw], ps0[:, 0:cw], BK0
                )
                nc.vector.tensor_scalar_add(
                    KT1D[0:DH, c0:c0 + cw], ps1[0:DH, 0:cw], BK1
                )
            nc.sync.dma_start(out=KT1D[DH:P, :], in_=KT1D[0:DH, :])

            # ---- Q projection (one 1024-wide column chunk) ----
            def q_proj(c0, cw):
                ps0 = ps.tile([P, 1024], F32, tag="sp", bufs=2, name=f"qps0_{c0}")
                ps1 = ps.tile([DH, 1024], F32, tag="ctx01", bufs=1, name=f"qps1_{c0}")
                for kt in range(NKT):
                    for (h0, hw) in _chunks(cw, 512):
                        nc.tensor.matmul(
                            ps0[:, h0:h0 + hw],
                            lhsT=WQ[:, kt, 0:P],
                            rhs=XQ[kt][:, c0 + h0:c0 + h0 + hw],
                            start=(kt == 0), stop=(kt == NKT - 1),
                        )
                for kt in range(NKT):
                    for (h0, hw) in _chunks(cw, 512):
                        nc.tensor.matmul(
                            ps1[:, h0:h0 + hw],
                            lhsT=WQ[:, kt, P:DQ],
                            rhs=XQ[kt][:, c0 + h0:c0 + h0 + hw],
                            start=(kt == 0), stop=(kt == NKT - 1),
                        )
                nc.vector.tensor_scalar_add(
                    QT0[:, c0:c0 + cw], ps0[:, 0:cw], BQ0
                )
                nc.vector.tensor_scalar_add(
                    QT1D[0:DH, c0:c0 + cw], ps1[0:DH, 0:cw], BQ1
                )
                nc.sync.dma_start(
                    out=QT1D[DH:P, c0:c0 + cw], in_=QT1D[0:DH, c0:c0 + cw]
                )

            # only the first 512 Q columns are projected up front; each later
            # 512-piece is emitted just before the attention chunk that needs
            # it, so its matmuls fill the previous chunk's PE gaps
            q_proj(0, 1024)

            # ---- V projection ----
            for t in range(T):
                vps = ps.tile([P, DQ], F32, tag="opo", bufs=1, name=f"vps{t}")
                for kt in range(NKT):
                    nc.tensor.matmul(
                        vps,
                        lhsT=XV[kt][:, t * P:(t + 1) * P],
                        rhs=WV[:, kt, :],
                        start=(kt == 0), stop=(kt == NKT - 1),
                    )
                vview = VP[:, t, :].rearrange("p (h c) -> p h c", c=P)
                nc.vector.tensor_add(
                    vview[:, :, 0:DH],
                    vps.rearrange("p (h d) -> p h d", d=DH),
                    BV[:, :].rearrange("p (h d) -> p h d", d=DH),
                )
                nc.vector.tensor_scalar_mul(
                    vview[:, :, 0:DH], vview[:, :, 0:DH], VM[:, t:t + 1]
                )
                nc.vector.tensor_copy(
                    vview[:, :, DH:DH + 1],
                    VM[:, t:t + 1].to_broadcast([P, HPG, 1]),
                )

            # ---- attention + output projection, per query chunk ----
            def attn_norm(ctxp, col, dst, cw, uid):
                # denominator -> SBUF -> reciprocal -> partition-broadcast,
                # then scale ctx rows 0:64 straight out of PSUM
                dn = norm.tile([1, 512], F32, tag="dn", name=f"dn{uid}")
                nc.vector.tensor_copy(
                    dn[:, 0:cw], ctxp[DH:DH + 1, col:col + cw]
                )
                rc = norm.tile([1, 512], F32, tag="rc", name=f"rc{uid}")
                nc.vector.reciprocal_approx_fast(rc[:, 0:cw], dn[:, 0:cw])
                bc = norm.tile([DH, 512], F32, tag="bc", name=f"bc{uid}")
                nc.gpsimd.partition_broadcast(bc[:, 0:cw], rc[:, 0:cw])
                nc.vector.tensor_mul(
                    dst[:, 0:cw], ctxp[0:DH, col:col + cw], bc[:, 0:cw]
                )

            def h01_scores_exp(ci, c0, cw, t):
                sp = ps.tile([P, 1024], F32, tag="sp", bufs=2, name=f"sp{ci}_{t}")
                nc.tensor.matmul(
                    sp[:, 0:cw],
                    lhsT=KT0[0:DH, t * P:(t + 1) * P],
                    rhs=QT0[0:DH, c0:c0 + cw],
                    start=True, stop=True,
                )
                nc.tensor.matmul(
                    sp[:, 512:512 + cw],
                    lhsT=KT0[DH:P, t * P:(t + 1) * P],
                    rhs=QT0[DH:P, c0:c0 + cw],
                    start=True, stop=True,
                )
                es = espool.tile([P, 1024], BF, tag="es", name=f"es{ci}_{t}")
                nc.scalar.activation(
                    es, sp, AFT.Exp, bias=0.0, scale=1.0 / math.sqrt(DH),
                )
                return es

            # O-proj piece: one (m-tile, n-chunk) matmul pair + evict + store.
            # Pieces for chunk c are emitted one-per-t-slot inside chunk c+1's
            # loop so the PE never drains a big O-proj block while ACT starves.
            def opo_piece(m, n0, nw, CTX01, CTX2, mi, last, evict_dve):
                def emit():
                    po = ps.tile(
                        [P, 384], F32,
                        tag="sp" if last else "opo",
                        bufs=2 if last else 1,
                        name=f"po{m}_{n0}",
                    )
                    nc.tensor.matmul(
                        po[:, 0:nw],
                        lhsT=CTX01[:, mi * P:(mi + 1) * P],
                        rhs=WO0[:, n0:n0 + nw],
                        start=True, stop=False,
                    )
                    nc.tensor.matmul(
                        po[:, 0:nw],
                        lhsT=CTX2[:, mi * P:(mi + 1) * P],
                        rhs=WO2[:, n0:n0 + nw],
                        start=False, stop=True,
                    )
                    po_sb = osb.tile([P, 384], BF, tag="posb", name=f"posb{m}_{n0}")
                    if evict_dve:
                        nc.vector.tensor_copy(po_sb[:, 0:nw], po[:, 0:nw])
                    else:
                        nc.scalar.copy(po_sb[:, 0:nw], po[:, 0:nw])
                    nc.sync.dma_start(
                        out=out[m * P:(m + 1) * P, n0:n0 + nw], in_=po_sb[:, 0:nw]
                    )
                return emit

            pend = []
            pre = {}
            NACH = len(ACH)
            for ci, (c0, cw) in enumerate(ACH):
                if ci == 2:
                    q_proj(1024, 1024)
                ctxp = ps.tile([P, 1024], F32, tag="ctx01", bufs=1, name=f"ctx{ci}")
                ctx2p = ps.tile([P, 512], F32, tag="ctx2", bufs=1, name=f"ctx2_{ci}")
                pieces = pend
                pend = []
                for t in range(T):
                    es = pre.pop((ci, t), None)
                    if es is None:
                        es = h01_scores_exp(ci, c0, cw, t)
                    nc.tensor.matmul(
                        ctxp[:, 0:cw],
                        lhsT=VP[:, t, 0:P],
                        rhs=es[:, 0:cw],
                        start=(t == 0), stop=(t == T - 1),
                    )
                    nc.tensor.matmul(
                        ctxp[:, 512:512 + cw],
                        lhsT=VP[:, t, P:2 * P],
                        rhs=es[:, 512:512 + cw],
                        start=(t == 0), stop=(t == T - 1),
                    )
                    if t >= 1 and pieces:
                        pieces.pop(0)()
                for f in pieces:
                    f()
                # h0/h1 norms: one wide copy/recip/broadcast for both heads
                CTX01 = ctxs.tile([P, 512], BF, tag="ctx01", name=f"CTX01_{ci}")
                CTX2 = ctxs.tile([DH, 512], BF, tag="ctx2", name=f"CTX2_{ci}")
                dn = norm.tile([1, 1024], F32, tag="dn01", name=f"dn01_{ci}")
                nc.vector.tensor_copy(dn, ctxp[DH:DH + 1, 0:1024])
                rc = norm.tile([1, 1024], F32, tag="rc01", name=f"rc01_{ci}")
                nc.vector.reciprocal_approx_fast(rc, dn)
                bc = norm.tile([DH, 1024], F32, tag="bc01", name=f"bc01_{ci}")
                nc.gpsimd.partition_broadcast(bc, rc)
                nc.vector.tensor_mul(
                    CTX01[0:DH, :], ctxp[0:DH, 0:512], bc[:, 0:512]
                )
                nc.vector.tensor_mul(
                    CTX01[DH:P, :], ctxp[0:DH, 512:1024], bc[:, 512:1024]
                )
                # prefetch the next chunk's first scores+exp so ACT stays fed
                # across the chunk boundary
                if ci + 1 < NACH:
                    nco = ACH[ci + 1][0]
                    for tp in range(0):
                        pre[(ci + 1, tp)] = h01_scores_exp(ci + 1, nco, 512, tp)
                # head 2: key-tile pairs in row groups 0:64 / 64:128
                t = 0
                while t < T:
                    ln = min(2, T - t)
                    sp = ps.tile([P, 1024], F32, tag="sp", bufs=2, name=f"sp2_{ci}_{t}")
                    nc.tensor.matmul(
                        sp[:, 0:cw],
                        lhsT=KT1D[0:DH, t * P:(t + 1) * P],
                        rhs=QT1D[0:DH, c0:c0 + cw],
                        start=True, stop=True,
                    )
                    if ln == 2:
                        nc.tensor.matmul(
                            sp[:, 512:512 + cw],
                            lhsT=KT1D[DH:P, (t + 1) * P:(t + 2) * P],
                            rhs=QT1D[DH:P, c0:c0 + cw],
                            start=True, stop=True,
                        )
                    es = espool.tile([P, 1024], BF, tag="es", name=f"es2_{ci}_{t}")
                    nc.scalar.activation(
                        es[:, 0:ln * 512], sp[:, 0:ln * 512],
                        AFT.Exp, bias=0.0, scale=1.0 / math.sqrt(DH),
                    )
                    for i in range(ln):
                        nc.tensor.matmul(
                            ctx2p[:, 0:cw],
                            lhsT=VP[:, t + i, 2 * P:3 * P],
                            rhs=es[:, i * 512:i * 512 + cw],
                            start=(t + i == 0), stop=(t + i == T - 1),
                        )
                    t += ln
                # h2 norm (sole reader of ctx2p; h01 slot already released)
                attn_norm(ctx2p, 0, CTX2, cw, f"{ci}_2")
                # queue this chunk's O-proj pieces; the last chunk emits its
                # own (on the freed scores slots so rounds pipeline)
                last = ci == NACH - 1
                for mi in range(cw // P):
                    m = c0 // P + mi
                    for ni, (n0, nw) in enumerate(NCH):
                        pend.append(opo_piece(
                            m, n0, nw, CTX01, CTX2, mi, last,
                            (mi + ni) % 2 == 0 or not last,
                        ))
                for f in pend:
                    f()
                pend = []
    nc.compile()
    return nc


def _get_prog(KP):
    if KP not in _prog_cache:
        _prog_cache[KP] = _build_nc(KP)
    return _prog_cache[KP]


def _run(inputs, trace=False):
    import ml_dtypes
    from concourse.bass_utils import run_bass_kernel_spmd

    BF = ml_dtypes.bfloat16

    def _warr(w):
        # [768, DQ] -> [128, NKT*DQ], contiguous per partition
        a = w.reshape(NKT, P, DQ).transpose(1, 0, 2).reshape(P, NKT * DQ)
        return np.ascontiguousarray(a).astype(BF)

    query = np.asarray(inputs["query"], dtype=np.float32)
    key = np.asarray(inputs["key"], dtype=np.float32)
    value = np.asarray(inputs["value"], dtype=np.float32)
    mask = np.asarray(inputs["mask"])
    Wq = np.asarray(inputs["Wq"], dtype=np.float32)
    bq = np.asarray(inputs["bq"], dtype=np.float32)
    Wk = np.asarray(inputs["Wk"], dtype=np.float32)
    bk = np.asarray(inputs["bk"], dtype=np.float32)
    Wv = np.asarray(inputs["Wv"], dtype=np.float32)
    bv = np.asarray(inputs["bv"], dtype=np.float32)
    Wo = np.asarray(inputs["Wo"], dtype=np.float32)
    bo = np.asarray(inputs["bo"], dtype=np.float32)

    idx = [np.nonzero(mask[b, 0, 0] != 0)[0] for b in range(B)]
    keff = [len(i) for i in idx]
    KP = max(P, ((max(keff) + P - 1) // P) * P)
    T = KP // P

    nc = _get_prog(KP)

    per_batch = {}
    for b in range(B):
        xqT = np.ascontiguousarray(query[b].T).astype(BF)
        xkT = np.zeros((DM, KP), dtype=BF)
        xkT[:, :keff[b]] = key[b][idx[b]].T.astype(BF)
        xvT = np.zeros((DM, KP), dtype=BF)
        xvT[:, :keff[b]] = value[b][idx[b]].T.astype(BF)
        vmf = np.zeros((KP,), dtype=np.float32)
        vmf[:keff[b]] = 1.0
        vm2 = np.ascontiguousarray(vmf.reshape(T, P).T)  # [128, T]
        per_batch[b] = (xqT, xkT, xvT, vm2)

    in_maps = []
    for core in range(NCORES):
        b, g = core // G, core % G
        xqT, xkT, xvT, vm2 = per_batch[b]
        sl = slice(g * DQ, (g + 1) * DQ)
        in_maps.append({
            "xqT": xqT,
            "xkT": xkT,
            "xvT": xvT,
            "wq": _warr(Wq[:, sl]),
            "wk": _warr(Wk[:, sl]),
            "wv": _warr(Wv[:, sl]),
            "wo": np.ascontiguousarray(Wo[sl, :]).astype(BF),
            "bq": np.ascontiguousarray(bq[sl].reshape(DQ, 1)),
            "bk": np.ascontiguousarray(bk[sl].reshape(DQ, 1)),
            "bv": np.ascontiguousarray(bv[sl].reshape(1, DQ)),
            "vm": vm2,
        })

    res = run_bass_kernel_spmd(nc, in_maps, list(range(NCORES)), trace=trace)

    outp = np.zeros((B, S, DM), dtype=np.float32)
    for core in range(NCORES):
        outp[core // G] += np.asarray(res.results[core]["out"], dtype=np.float32)
    outp += bo.reshape(1, 1, DM)
    return outp, res


def kernel(**inputs) -> np.ndarray:
    out, _ = _run(inputs, trace=False)
    return out


if __name__ == "__main__":
    nc = _build_nc(1152)
    print("build OK")


# revision 51
# speedup vs baseline: 1.1410x; 1.1410x over previous
"""Multi-head attention (B=2, S=2048, d_model=768, H=12) on 8 TRN2 NeuronCores.

Sharding: 2-way data parallel over batch x 4-way tensor parallel over heads
(3 heads / 192-wide d_model slice per core). Host compacts masked keys away
(gather of unmasked key/value rows), pads to a 128 multiple, and passes a 0/1
validity vector; softmax needs no mask handling on device (pad keys get V=0
and a 0 in the denominator ones-column). Per core:

    Q^T [192,2048], K^T [192,KP] via projections (dq on partitions)
    V   [KP,192] natural layout, x3 per-head [V_h | valid] blocks
    per head: scores^T[k,q] = K_h^T.T @ Q_h^T ; es = exp(s/8) on ACT
              ctx'^T[65,q] += [V_h|valid].T @ es  (row 64 = denominator)
              ctx = ctx * recip(denom) (DVE + gpsimd partition_broadcast)
    out_partial[2048,768] = ctx^T.T @ Wo_g (bf16), summed on host + bo.

v2 layout decisions (from baseline trace analysis):
  - ACT runs exp only during attention; projection bias-evicts happen on ACT
    while it is otherwise idle (before the first exp); norm + O-evict on DVE.
  - PSUM: sp tag 2x[128,1024] (scores dbuf + proj g0), ctx tag 1x[128,1536]
    (3 per-head banks + proj g1), opo tag 1x[128,384] (O-proj + V-proj).
  - heads 0/1 scores pair in PE row groups 0:64/64:128 (concurrent matmuls);
    head 2 pairs key-tiles t/t+1 the same way via partition-duplicated
    KT1D/QT1D (dup done by SBUF->SBUF DMA).
  - DMA order: weights, xk, xq half 0, xv, xq half 1 -> compute starts ~2us.
  - out partials are bf16 (halves exit traffic), summed f32 on host.
"""

import math

import numpy as np

B = 2
S = 2048
DM = 768
H = 12
DH = 64
G = 4              # head-group (tensor-parallel) degree
HPG = H // G       # heads per core
DQ = HPG * DH      # 192 d_model slice per core
NCORES = 8
P = 128
NKT = DM // P      # 6 contraction tiles for projections

_prog_cache = {}


def _chunks(total, step):
    out = []
    o = 0
    while o < total:
        w = min(step, total - o)
        out.append((o, w))
        o += w
    return out


def _build_nc(KP):
    import concourse.bass as bass
    import concourse.mybir as mybir
    import concourse.tile as tile
    from concourse import bacc

    F32 = mybir.dt.float32
    BF = mybir.dt.bfloat16
    AFT = mybir.ActivationFunctionType

    T = KP // P
    KCH = _chunks(KP, 1024)
    QPCH = _chunks(S, 1024)      # projection chunks for Q
    ACH = _chunks(S, 512)        # attention query chunks
    NCH = _chunks(DM, 384)       # O-proj output chunks (1 PSUM bank each)

    nc = bacc.Bacc(None, target_bir_lowering=False)
    xqT = nc.declare_dram_parameter("xqT", [DM, S], BF, isOutput=False)
    xkT = nc.declare_dram_parameter("xkT", [DM, KP], BF, isOutput=False)
    xvT = nc.declare_dram_parameter("xvT", [DM, KP], BF, isOutput=False)
    # host pre-arranges projection weights to [128, ...] so the upload is
    # one contiguous stripe per partition (the old "(kt p) m -> p kt m"
    # rearrange fragmented into 768 tiny descriptors)
    wq = nc.declare_dram_parameter("wq", [P, NKT * DQ], BF, isOutput=False)
    wk = nc.declare_dram_parameter("wk", [P, NKT * DQ], BF, isOutput=False)
    wv = nc.declare_dram_parameter("wv", [P, NKT * DQ], BF, isOutput=False)
    wo = nc.declare_dram_parameter("wo", [DQ, DM], BF, isOutput=False)
    bq = nc.declare_dram_parameter("bq", [DQ, 1], F32, isOutput=False)
    bk = nc.declare_dram_parameter("bk", [DQ, 1], F32, isOutput=False)
    bv = nc.declare_dram_parameter("bv", [1, DQ], F32, isOutput=False)
    vm = nc.declare_dram_parameter("vm", [P, T], F32, isOutput=False)
    out = nc.declare_dram_parameter("out", [S, DM], BF, isOutput=True)

    with tile.TileContext(nc) as tc:
        with (
            tc.tile_pool(name="persist", bufs=1) as persist,
            tc.tile_pool(name="acts", bufs=6) as acts,
            tc.tile_pool(name="es", bufs=6) as espool,
            tc.tile_pool(name="norm", bufs=4) as norm,
            tc.tile_pool(name="osb", bufs=6) as osb,
            tc.tile_pool(name="ctxs", bufs=2) as ctxs,
            tc.tile_pool(name="ps", bufs=1, space="PSUM") as ps,
        ):
            # ---- DMAs in startup-critical order: the sync engine generates
            # descriptors serially (~0.7us each), so K-proj inputs go first
            WK = persist.tile([P, NKT, DQ], BF, tag="WK")
            WQ = persist.tile([P, NKT, DQ], BF, tag="WQ")
            WV = persist.tile([P, NKT, DQ], BF, tag="WV")
            BK0 = persist.tile([P, 1], F32, tag="BK0")
            BK1 = persist.tile([DH, 1], F32, tag="BK1")
            nc.sync.dma_start(out=WK, in_=wk[:, :].rearrange("p (kt m) -> p kt m", m=DQ))
            nc.sync.dma_start(out=BK0, in_=bk[0:P, :])
            nc.sync.dma_start(out=BK1, in_=bk[P:DQ, :])
            XH = _chunks(KP, (KP + 255) // 256 * 128)
            XK = []
            for kt in range(NKT):
                xt = acts.tile([P, KP], BF, tag="xk", name=f"xk{kt}")
                for (h0, hw) in XH:
                    nc.sync.dma_start(
                        out=xt[:, h0:h0 + hw],
                        in_=xkT[kt * P:(kt + 1) * P, h0:h0 + hw],
                    )
                XK.append(xt)
            BQ0 = persist.tile([P, 1], F32, tag="BQ0")
            BQ1 = persist.tile([DH, 1], F32, tag="BQ1")
            nc.sync.dma_start(out=WQ, in_=wq[:, :].rearrange("p (kt m) -> p kt m", m=DQ))
            nc.sync.dma_start(out=BQ0, in_=bq[0:P, :])
            nc.sync.dma_start(out=BQ1, in_=bq[P:DQ, :])
            XQ = []
            for kt in range(NKT):
                xt = acts.tile([P, S], BF, tag="xq", name=f"xq{kt}")
                nc.sync.dma_start(out=xt[:, 0:1024], in_=xqT[kt * P:(kt + 1) * P, 0:1024])
                XQ.append(xt)
            nc.sync.dma_start(out=WV, in_=wv[:, :].rearrange("p (kt m) -> p kt m", m=DQ))
            BV = persist.tile([P, DQ], F32, tag="BV")
            nc.sync.dma_start(out=BV, in_=bv[:, :].to_broadcast([P, DQ]))
            VM = persist.tile([P, T], F32, tag="VM")
            nc.sync.dma_start(out=VM, in_=vm[:, :])
            XV = []
            for kt in range(NKT):
                xt = acts.tile([P, KP], BF, tag="xv", name=f"xv{kt}")
                for (h0, hw) in XH:
                    nc.sync.dma_start(
                        out=xt[:, h0:h0 + hw],
                        in_=xvT[kt * P:(kt + 1) * P, h0:h0 + hw],
                    )
                XV.append(xt)
            WO0 = persist.tile([P, DM], BF, tag="WO0")   # wo rows 0:128 (h0,h1)
            WO2 = persist.tile([DH, DM], BF, tag="WO2")  # wo rows 128:192 (h2)
            nc.sync.dma_start(out=WO0, in_=wo[0:P, :])
            nc.sync.dma_start(out=WO2, in_=wo[P:DQ, :])
            for kt in range(NKT):
                nc.sync.dma_start(
                    out=XQ[kt][:, 1024:S], in_=xqT[kt * P:(kt + 1) * P, 1024:S]
                )

            # ---- persistent activations ----
            KT0 = persist.tile([P, KP], BF, tag="KT0")    # heads 0,1 (dq 0:128)
            KT1D = persist.tile([P, KP], BF, tag="KT1D")  # head 2, duplicated rows
            QT0 = persist.tile([P, S], BF, tag="QT0")
            QT1D = persist.tile([P, S], BF, tag="QT1D")
            # ---- PE warm-up: dependency-free matmuls run in the DMA-wait
            # shadow; >3.4us of continuous PE activity trips the HAM clock
            # gate to 8/8 before K-proj starts ----
            WUP = persist.tile([P, P], BF, tag="WUP")
            nc.vector.memset(WUP, 0.0)
            wps = ps.tile([P, P], F32, tag="opo", bufs=1, name="warmup_ps")
            for _w in range(40):
                nc.tensor.matmul(
                    wps, lhsT=WUP, rhs=WUP, start=True, stop=True,
                )

            # V blocks padded to 128 cols (cols 0:64 V, 64 ones, 65:128 zero)
            VP = persist.tile([P, T, HPG * P], BF, tag="VP")
            nc.vector.memset(VP, 0.0)

            # ---- K projection ----
            for (c0, cw) in KCH:
                ps0 = ps.tile([P, 1024], F32, tag="sp", bufs=2, name=f"kps0_{c0}")
                ps1 = ps.tile([DH, 1024], F32, tag="ctx01", bufs=1, name=f"kps1_{c0}")
                for kt in range(NKT):
                    for (h0, hw) in _chunks(cw, 512):
                        nc.tensor.matmul(
                            ps0[:, h0:h0 + hw],
                            lhsT=WK[:, kt, 0:P],
                            rhs=XK[kt][:, c0 + h0:c0 + h0 + hw],
                            start=(kt == 0), stop=(kt == NKT - 1),
                        )
                for kt in range(NKT):
                    for (h0, hw) in _chunks(cw, 512):
                        nc.tensor.matmul(
                            ps1[:, h0:h0 + hw],
                            lhsT=WK[:, kt, P:DQ],
                            rhs=XK[kt][:, c0 + h0:c0 + h0 + hw],
                            start=(kt == 0), stop=(kt == NKT - 1),
                        )
                nc.vector.tensor_scalar_add(
                    KT0[:, c0:c0 + cw], ps0[:, 0:cw], BK0
                )
                nc.vector.tensor_scalar_add(
                    KT1D[0:DH, c0:c0 + cw], ps1[0:DH, 0:cw], BK1
                )
            nc.sync.dma_start(out=KT1D[DH:P, :], in_=KT1D[0:DH, :])

            # ---- Q projection (one 1024-wide column chunk) ----
            def q_proj(c0, cw):
                ps0 = ps.tile([P, 1024], F32, tag="sp", bufs=2, name=f"qps0_{c0}")
                ps1 = ps.tile([DH, 1024], F32, tag="ctx01", bufs=1, name=f"qps1_{c0}")
                for kt in range(NKT):
                    for (h0, hw) in _chunks(cw, 512):
                        nc.tensor.matmul(
                            ps0[:, h0:h0 + hw],
                            lhsT=WQ[:, kt, 0:P],
                            rhs=XQ[kt][:, c0 + h0:c0 + h0 + hw],
                            start=(kt == 0), stop=(kt == NKT - 1),
                        )
                for kt in range(NKT):
                    for (h0, hw) in _chunks(cw, 512):
                        nc.tensor.matmul(
                            ps1[:, h0:h0 + hw],
                            lhsT=WQ[:, kt, P:DQ],
                            rhs=XQ[kt][:, c0 + h0:c0 + h0 + hw],
                            start=(kt == 0), stop=(kt == NKT - 1),
                        )
                nc.vector.tensor_scalar_add(
                    QT0[:, c0:c0 + cw], ps0[:, 0:cw], BQ0
                )
                nc.vector.tensor_scalar_add(
                    QT1D[0:DH, c0:c0 + cw], ps1[0:DH, 0:cw], BQ1
                )
                nc.sync.dma_start(
                    out=QT1D[DH:P, c0:c0 + cw], in_=QT1D[0:DH, c0:c0 + cw]
                )

            # only the first 512 Q columns are projected up front; each later
            # 512-piece is emitted just before the attention chunk that needs
            # it, so its matmuls fill the previous chunk's PE gaps
            q_proj(0, 512)

            # ---- V projection ----
            for t in range(T):
                vps = ps.tile([P, DQ], F32, tag="opo", bufs=1, name=f"vps{t}")
                for kt in range(NKT):
                    nc.tensor.matmul(
                        vps,
                        lhsT=XV[kt][:, t * P:(t + 1) * P],
                        rhs=WV[:, kt, :],
                        start=(kt == 0), stop=(kt == NKT - 1),
                    )
                vview = VP[:, t, :].rearrange("p (h c) -> p h c", c=P)
                nc.vector.tensor_add(
                    vview[:, :, 0:DH],
                    vps.rearrange("p (h d) -> p h d", d=DH),
                    BV[:, :].rearrange("p (h d) -> p h d", d=DH),
                )
                nc.vector.tensor_scalar_mul(
                    vview[:, :, 0:DH], vview[:, :, 0:DH], VM[:, t:t + 1]
                )
                nc.vector.tensor_copy(
                    vview[:, :, DH:DH + 1],
                    VM[:, t:t + 1].to_broadcast([P, HPG, 1]),
                )

            # ---- attention + output projection, per query chunk ----
            def attn_norm(ctxp, col, dst, cw, uid):
                # denominator -> SBUF -> reciprocal -> partition-broadcast,
                # then scale ctx rows 0:64 straight out of PSUM
                dn = norm.tile([1, 512], F32, tag="dn", name=f"dn{uid}")
                nc.vector.tensor_copy(
                    dn[:, 0:cw], ctxp[DH:DH + 1, col:col + cw]
                )
                rc = norm.tile([1, 512], F32, tag="rc", name=f"rc{uid}")
                nc.vector.reciprocal_approx_fast(rc[:, 0:cw], dn[:, 0:cw])
                bc = norm.tile([DH, 512], F32, tag="bc", name=f"bc{uid}")
                nc.gpsimd.partition_broadcast(bc[:, 0:cw], rc[:, 0:cw])
                nc.vector.tensor_mul(
                    dst[:, 0:cw], ctxp[0:DH, col:col + cw], bc[:, 0:cw]
                )

            def h01_scores_exp(ci, c0, cw, t):
                sp = ps.tile([P, 1024], F32, tag="sp", bufs=2, name=f"sp{ci}_{t}")
                nc.tensor.matmul(
                    sp[:, 0:cw],
                    lhsT=KT0[0:DH, t * P:(t + 1) * P],
                    rhs=QT0[0:DH, c0:c0 + cw],
                    start=True, stop=True,
                )
                nc.tensor.matmul(
                    sp[:, 512:512 + cw],
                    lhsT=KT0[DH:P, t * P:(t + 1) * P],
                    rhs=QT0[DH:P, c0:c0 + cw],
                    start=True, stop=True,
                )
                es = espool.tile([P, 1024], BF, tag="es", name=f"es{ci}_{t}")
                nc.scalar.activation(
                    es, sp, AFT.Exp, bias=0.0, scale=1.0 / math.sqrt(DH),
                )
                return es

            # O-proj piece: one (m-tile, n-chunk) matmul pair + evict + store.
            # Pieces for chunk c are emitted one-per-t-slot inside chunk c+1's
            # loop so the PE never drains a big O-proj block while ACT starves.
            def opo_piece(m, n0, nw, CTX01, CTX2, mi, last, evict_dve):
                def emit():
                    po = ps.tile(
                        [P, 384], F32,
                        tag="sp" if last else "opo",
                        bufs=2 if last else 1,
                        name=f"po{m}_{n0}",
                    )
                    nc.tensor.matmul(
                        po[:, 0:nw],
                        lhsT=CTX01[:, mi * P:(mi + 1) * P],
                        rhs=WO0[:, n0:n0 + nw],
                        start=True, stop=False,
                    )
                    nc.tensor.matmul(
                        po[:, 0:nw],
                        lhsT=CTX2[:, mi * P:(mi + 1) * P],
                        rhs=WO2[:, n0:n0 + nw],
                        start=False, stop=True,
                    )
                    po_sb = osb.tile([P, 384], BF, tag="posb", name=f"posb{m}_{n0}")
                    if evict_dve:
                        nc.vector.tensor_copy(po_sb[:, 0:nw], po[:, 0:nw])
                    else:
                        nc.scalar.copy(po_sb[:, 0:nw], po[:, 0:nw])
                    nc.sync.dma_start(
                        out=out[m * P:(m + 1) * P, n0:n0 + nw], in_=po_sb[:, 0:nw]
                    )
                return emit

            pend = []
            pre = {}
            NACH = len(ACH)
            for ci, (c0, cw) in enumerate(ACH):
                if ci >= 1:
                    q_proj(ci * 512, 512)
                ctxp = ps.tile([P, 1024], F32, tag="ctx01", bufs=1, name=f"ctx{ci}")
                ctx2p = ps.tile([P, 512], F32, tag="ctx2", bufs=1, name=f"ctx2_{ci}")
                pieces = pend
                pend = []
                for t in range(T):
                    es = pre.pop((ci, t), None)
                    if es is None:
                        es = h01_scores_exp(ci, c0, cw, t)
                    nc.tensor.matmul(
                        ctxp[:, 0:cw],
                        lhsT=VP[:, t, 0:P],
                        rhs=es[:, 0:cw],
                        start=(t == 0), stop=(t == T - 1),
                    )
                    nc.tensor.matmul(
                        ctxp[:, 512:512 + cw],
                        lhsT=VP[:, t, P:2 * P],
                        rhs=es[:, 512:512 + cw],
                        start=(t == 0), stop=(t == T - 1),
                    )
                    if t >= 1 and pieces:
                        pieces.pop(0)()
                for f in pieces:
                    f()
                # h0/h1 norms: one wide copy/recip/broadcast for both heads
                CTX01 = ctxs.tile([P, 512], BF, tag="ctx01", name=f"CTX01_{ci}")
                CTX2 = ctxs.tile([DH, 512], BF, tag="ctx2", name=f"CTX2_{ci}")
                dn = norm.tile([1, 1024], F32, tag="dn01", name=f"dn01_{ci}")
                nc.vector.tensor_copy(dn, ctxp[DH:DH + 1, 0:1024])
                rc = norm.tile([1, 1024], F32, tag="rc01", name=f"rc01_{ci}")
                nc.vector.reciprocal_approx_fast(rc, dn)
                bc = norm.tile([DH, 1024], F32, tag="bc01", name=f"bc01_{ci}")
                nc.gpsimd.partition_broadcast(bc, rc)
                nc.vector.tensor_mul(
                    CTX01[0:DH, :], ctxp[0:DH, 0:512], bc[:, 0:512]
                )
                nc.vector.tensor_mul(
                    CTX01[DH:P, :], ctxp[0:DH, 512:1024], bc[:, 512:1024]
                )
                # prefetch the next chunk's first scores+exp so ACT stays fed
                # across the chunk boundary
                if ci + 1 < NACH:
                    nco = ACH[ci + 1][0]
                    for tp in range(0):
                        pre[(ci + 1, tp)] = h01_scores_exp(ci + 1, nco, 512, tp)
                # head 2: key-tile pairs in row groups 0:64 / 64:128
                t = 0
                while t < T:
                    ln = min(2, T - t)
                    sp = ps.tile([P, 1024], F32, tag="sp", bufs=2, name=f"sp2_{ci}_{t}")
                    nc.tensor.matmul(
                        sp[:, 0:cw],
                        lhsT=KT1D[0:DH, t * P:(t + 1) * P],
                        rhs=QT1D[0:DH, c0:c0 + cw],
                        start=True, stop=True,
                    )
                    if ln == 2:
                        nc.tensor.matmul(
                            sp[:, 512:512 + cw],
                            lhsT=KT1D[DH:P, (t + 1) * P:(t + 2) * P],
                            rhs=QT1D[DH:P, c0:c0 + cw],
                            start=True, stop=True,
                        )
                    es = espool.tile([P, 1024], BF, tag="es", name=f"es2_{ci}_{t}")
                    nc.scalar.activation(
                        es[:, 0:ln * 512], sp[:, 0:ln * 512],
                        AFT.Exp, bias=0.0, scale=1.0 / math.sqrt(DH),
                    )
                    for i in range(ln):
                        nc.tensor.matmul(
                            ctx2p[:, 0:cw],
                            lhsT=VP[:, t + i, 2 * P:3 * P],
                            rhs=es[:, i * 512:i * 512 + cw],
                            start=(t + i == 0), stop=(t + i == T - 1),
                        )
                    t += ln
                # h2 norm (sole reader of ctx2p; h01 slot already released)
                attn_norm(ctx2p, 0, CTX2, cw, f"{ci}_2")
                # queue this chunk's O-proj pieces; the last chunk emits its
                # own (on the freed scores slots so rounds pipeline)
                last = ci == NACH - 1
                for mi in range(cw // P):
                    m = c0 // P + mi
                    for ni, (n0, nw) in enumerate(NCH):
                        pend.append(opo_piece(
                            m, n0, nw, CTX01, CTX2, mi, last,
                            (mi + ni) % 2 == 0 or not last,
                        ))
                for f in pend:
                    f()
                pend = []
    nc.compile()
    return nc


def _get_prog(KP):
    if KP not in _prog_cache:
        _prog_cache[KP] = _build_nc(KP)
    return _prog_cache[KP]


def _run(inputs, trace=False):
    import ml_dtypes
    from concourse.bass_utils import run_bass_kernel_spmd

    BF = ml_dtypes.bfloat16

    def _warr(w):
        # [768, DQ] -> [128, NKT*DQ], contiguous per partition
        a = w.reshape(NKT, P, DQ).transpose(1, 0, 2).reshape(P, NKT * DQ)
        return np.ascontiguousarray(a).astype(BF)

    query = np.asarray(inputs["query"], dtype=np.float32)
    key = np.asarray(inputs["key"], dtype=np.float32)
    value = np.asarray(inputs["value"], dtype=np.float32)
    mask = np.asarray(inputs["mask"])
    Wq = np.asarray(inputs["Wq"], dtype=np.float32)
    bq = np.asarray(inputs["bq"], dtype=np.float32)
    Wk = np.asarray(inputs["Wk"], dtype=np.float32)
    bk = np.asarray(inputs["bk"], dtype=np.float32)
    Wv = np.asarray(inputs["Wv"], dtype=np.float32)
    bv = np.asarray(inputs["bv"], dtype=np.float32)
    Wo = np.asarray(inputs["Wo"], dtype=np.float32)
    bo = np.asarray(inputs["bo"], dtype=np.float32)

    idx = [np.nonzero(mask[b, 0, 0] != 0)[0] for b in range(B)]
    keff = [len(i) for i in idx]
    KP = max(P, ((max(keff) + P - 1) // P) * P)
    T = KP // P

    nc = _get_prog(KP)

    per_batch = {}
    for b in range(B):
        xqT = np.ascontiguousarray(query[b].T).astype(BF)
        xkT = np.zeros((DM, KP), dtype=BF)
        xkT[:, :keff[b]] = key[b][idx[b]].T.astype(BF)
        xvT = np.zeros((DM, KP), dtype=BF)
        xvT[:, :keff[b]] = value[b][idx[b]].T.astype(BF)
        vmf = np.zeros((KP,), dtype=np.float32)
        vmf[:keff[b]] = 1.0
        vm2 = np.ascontiguousarray(vmf.reshape(T, P).T)  # [128, T]
        per_batch[b] = (xqT, xkT, xvT, vm2)

    in_maps = []
    for core in range(NCORES):
        b, g = core // G, core % G
        xqT, xkT, xvT, vm2 = per_batch[b]
        sl = slice(g * DQ, (g + 1) * DQ)
        in_maps.append({
            "xqT": xqT,
            "xkT": xkT,
            "xvT": xvT,
            "wq": _warr(Wq[:, sl]),
            "wk": _warr(Wk[:, sl]),
            "wv": _warr(Wv[:, sl]),
            "wo": np.ascontiguousarray(Wo[sl, :]).astype(BF),
            "bq": np.ascontiguousarray(bq[sl].reshape(DQ, 1)),
            "bk": np.ascontiguousarray(bk[sl].reshape(DQ, 1)),
            "bv": np.ascontiguousarray(bv[sl].reshape(1, DQ)),
            "vm": vm2,
        })

    res = run_bass_kernel_spmd(nc, in_maps, list(range(NCORES)), trace=trace)

    outp = np.zeros((B, S, DM), dtype=np.float32)
    for core in range(NCORES):
        outp[core // G] += np.asarray(res.results[core]["out"], dtype=np.float32)
    outp += bo.reshape(1, 1, DM)
    return outp, res


def kernel(**inputs) -> np.ndarray:
    out, _ = _run(inputs, trace=False)
    return out


if __name__ == "__main__":
    nc = _build_nc(1152)
    print("build OK")


# revision 53
# speedup vs baseline: 1.1811x; 1.0351x over previous
"""Multi-head attention (B=2, S=2048, d_model=768, H=12) on 8 TRN2 NeuronCores.

Sharding: 2-way data parallel over batch x 4-way tensor parallel over heads
(3 heads / 192-wide d_model slice per core). Host compacts masked keys away
(gather of unmasked key/value rows), pads to a 128 multiple, and passes a 0/1
validity vector; softmax needs no mask handling on device (pad keys get V=0
and a 0 in the denominator ones-column). Per core:

    Q^T [192,2048], K^T [192,KP] via projections (dq on partitions)
    V   [KP,192] natural layout, x3 per-head [V_h | valid] blocks
    per head: scores^T[k,q] = K_h^T.T @ Q_h^T ; es = exp(s/8) on ACT
              ctx'^T[65,q] += [V_h|valid].T @ es  (row 64 = denominator)
              ctx = ctx * recip(denom) (DVE + gpsimd partition_broadcast)
    out_partial[2048,768] = ctx^T.T @ Wo_g (bf16), summed on host + bo.

Layout decisions (from perfetto trace analysis of earlier revisions):
  - ACT runs exp only (its ~58us stream is the soft floor); projection bias
    evicts, softmax normalization and O-proj evicts run on DVE; the
    reciprocal's partition broadcast runs on the otherwise-idle GPSIMD.
  - PSUM (8 banks): "sp" 2x[128,1024] (scores double-buffer + proj g0 psums),
    "ctx01" 1x[128,1024] (h0/h1 ctx + denominators; also proj g1 psums),
    "ctx2" 1x[128,512] (h2 ctx), "opo" 1x[128,384] (O-proj + V-proj).
    Splitting h2's ctx from h0/h1's lets each slot release right after its
    own norm, so the next chunk's accumulation never stalls ~7us (the
    single-slot version re-throttled the PE clock gate every chunk).
  - heads 0/1 scores pair in PE row groups 0:64/64:128 (concurrent matmuls);
    head 2 pairs key-tiles t/t+1 the same way via partition-duplicated
    KT1D/QT1D (dup via SBUF->SBUF DMA). Reciprocal runs from SBUF only (the
    custom-DVE op mis-executes on a PSUM source).
  - 40 dependency-free warm-up matmuls run in the DMA-wait shadow to lift
    the HAM clock gate to 8/8 before K-proj starts.
  - DMA order: wk, xk halves, wq, xq half 0, wv, xv halves, wo, xq half 1;
    weights are host-prearranged to per-partition-contiguous stripes.
  - Q columns are projected in 512-wide pieces, each emitted just before the
    attention chunk that needs it; O-proj is emitted per chunk with its last
    chunk's po tiles on the freed scores slots so the tail pipelines.
  - out partials are bf16 (halves exit traffic), summed f32 on host.
  - fp8 (DoubleRow) was tried for Q/K projections and rejected: e4m3
    quantization noise (~5% rms) propagates essentially 1:1 to the output
    (random-sign signals give no sqrt-N averaging) -> rel err 5.8e-2.
"""

import math

import numpy as np

B = 2
S = 2048
DM = 768
H = 12
DH = 64
G = 4              # head-group (tensor-parallel) degree
HPG = H // G       # heads per core
DQ = HPG * DH      # 192 d_model slice per core
NCORES = 8
P = 128
NKT = DM // P      # 6 contraction tiles for projections

_prog_cache = {}


def _chunks(total, step):
    out = []
    o = 0
    while o < total:
        w = min(step, total - o)
        out.append((o, w))
        o += w
    return out


def _build_nc(KP):
    import concourse.bass as bass
    import concourse.mybir as mybir
    import concourse.tile as tile
    from concourse import bacc

    F32 = mybir.dt.float32
    BF = mybir.dt.bfloat16
    AFT = mybir.ActivationFunctionType

    T = KP // P
    KCH = _chunks(KP, 1024)
    QPCH = _chunks(S, 1024)      # projection chunks for Q
    ACH = _chunks(S, 512)        # attention query chunks
    NCH = _chunks(DM, 384)       # O-proj output chunks (1 PSUM bank each)

    nc = bacc.Bacc(None, target_bir_lowering=False)
    xqT = nc.declare_dram_parameter("xqT", [DM, S], BF, isOutput=False)
    xkT = nc.declare_dram_parameter("xkT", [DM, KP], BF, isOutput=False)
    xvT = nc.declare_dram_parameter("xvT", [DM, KP], BF, isOutput=False)
    # host pre-arranges projection weights to [128, ...] so the upload is
    # one contiguous stripe per partition (the old "(kt p) m -> p kt m"
    # rearrange fragmented into 768 tiny descriptors)
    wq = nc.declare_dram_parameter("wq", [P, NKT * DQ], BF, isOutput=False)
    wk = nc.declare_dram_parameter("wk", [P, NKT * DQ], BF, isOutput=False)
    wv = nc.declare_dram_parameter("wv", [P, NKT * DQ], BF, isOutput=False)
    wo = nc.declare_dram_parameter("wo", [DQ, DM], BF, isOutput=False)
    bq = nc.declare_dram_parameter("bq", [DQ, 1], F32, isOutput=False)
    bk = nc.declare_dram_parameter("bk", [DQ, 1], F32, isOutput=False)
    bv = nc.declare_dram_parameter("bv", [1, DQ], F32, isOutput=False)
    vm = nc.declare_dram_parameter("vm", [P, T], F32, isOutput=False)
    out = nc.declare_dram_parameter("out", [S, DM], BF, isOutput=True)

    with tile.TileContext(nc) as tc:
        with (
            tc.tile_pool(name="persist", bufs=1) as persist,
            tc.tile_pool(name="acts", bufs=6) as acts,
            tc.tile_pool(name="es", bufs=6) as espool,
            tc.tile_pool(name="norm", bufs=4) as norm,
            tc.tile_pool(name="osb", bufs=6) as osb,
            tc.tile_pool(name="ctxs", bufs=2) as ctxs,
            tc.tile_pool(name="ps", bufs=1, space="PSUM") as ps,
        ):
            # ---- DMAs in startup-critical order: the sync engine generates
            # descriptors serially (~0.7us each), so K-proj inputs go first
            WK = persist.tile([P, NKT, DQ], BF, tag="WK")
            WQ = persist.tile([P, NKT, DQ], BF, tag="WQ")
            WV = persist.tile([P, NKT, DQ], BF, tag="WV")
            BK0 = persist.tile([P, 1], F32, tag="BK0")
            BK1 = persist.tile([DH, 1], F32, tag="BK1")
            nc.sync.dma_start(out=WK, in_=wk[:, :].rearrange("p (kt m) -> p kt m", m=DQ))
            XH = _chunks(KP, (KP + 255) // 256 * 128)
            XK = []
            for kt in range(NKT):
                xt = acts.tile([P, KP], BF, tag="xk", name=f"xk{kt}")
                for (h0, hw) in XH:
                    nc.sync.dma_start(
                        out=xt[:, h0:h0 + hw],
                        in_=xkT[kt * P:(kt + 1) * P, h0:h0 + hw],
                    )
                XK.append(xt)
            BQ0 = persist.tile([P, 1], F32, tag="BQ0")
            BQ1 = persist.tile([DH, 1], F32, tag="BQ1")
            nc.sync.dma_start(out=WQ, in_=wq[:, :].rearrange("p (kt m) -> p kt m", m=DQ))
            XQ = []
            for kt in range(NKT):
                xt = acts.tile([P, S], BF, tag="xq", name=f"xq{kt}")
                nc.sync.dma_start(out=xt[:, 0:1024], in_=xqT[kt * P:(kt + 1) * P, 0:1024])
                XQ.append(xt)
            nc.sync.dma_start(out=BK0, in_=bk[0:P, :])
            nc.sync.dma_start(out=BK1, in_=bk[P:DQ, :])
            nc.sync.dma_start(out=BQ0, in_=bq[0:P, :])
            nc.sync.dma_start(out=BQ1, in_=bq[P:DQ, :])
            nc.sync.dma_start(out=WV, in_=wv[:, :].rearrange("p (kt m) -> p kt m", m=DQ))
            BV = persist.tile([P, DQ], F32, tag="BV")
            nc.sync.dma_start(out=BV, in_=bv[:, :].to_broadcast([P, DQ]))
            VM = persist.tile([P, T], F32, tag="VM")
            nc.sync.dma_start(out=VM, in_=vm[:, :])
            XV = []
            for kt in range(NKT):
                xt = acts.tile([P, KP], BF, tag="xv", name=f"xv{kt}")
                for (h0, hw) in XH:
                    nc.sync.dma_start(
                        out=xt[:, h0:h0 + hw],
                        in_=xvT[kt * P:(kt + 1) * P, h0:h0 + hw],
                    )
                XV.append(xt)
            WO0 = persist.tile([P, DM], BF, tag="WO0")   # wo rows 0:128 (h0,h1)
            WO2 = persist.tile([DH, DM], BF, tag="WO2")  # wo rows 128:192 (h2)
            nc.sync.dma_start(out=WO0, in_=wo[0:P, :])
            nc.sync.dma_start(out=WO2, in_=wo[P:DQ, :])
            for kt in range(NKT):
                nc.sync.dma_start(
                    out=XQ[kt][:, 1024:S], in_=xqT[kt * P:(kt + 1) * P, 1024:S]
                )

            # ---- persistent activations ----
            KT0 = persist.tile([P, KP], BF, tag="KT0")    # heads 0,1 (dq 0:128)
            KT1D = persist.tile([P, KP], BF, tag="KT1D")  # head 2, duplicated rows
            QT0 = persist.tile([P, S], BF, tag="QT0")
            QT1D = persist.tile([P, S], BF, tag="QT1D")
            # ---- PE warm-up: dependency-free matmuls run in the DMA-wait
            # shadow; >3.4us of continuous PE activity trips the HAM clock
            # gate to 8/8 before K-proj starts ----
            WUP = persist.tile([P, P], BF, tag="WUP")
            nc.vector.memset(WUP, 0.0)
            wps = ps.tile([P, P], F32, tag="opo", bufs=1, name="warmup_ps")
            for _w in range(40):
                nc.tensor.matmul(
                    wps, lhsT=WUP, rhs=WUP, start=True, stop=True,
                )

            # V blocks padded to 128 cols (cols 0:64 V, 64 ones, 65:128 zero)
            VP = persist.tile([P, T, HPG * P], BF, tag="VP")
            nc.vector.memset(VP, 0.0)

            # ---- K projection ----
            for (c0, cw) in KCH:
                ps0 = ps.tile([P, 1024], F32, tag="sp", bufs=2, name=f"kps0_{c0}")
                ps1 = ps.tile([DH, 1024], F32, tag="ctx01", bufs=1, name=f"kps1_{c0}")
                for kt in range(NKT):
                    for (h0, hw) in _chunks(cw, 512):
                        nc.tensor.matmul(
                            ps0[:, h0:h0 + hw],
                            lhsT=WK[:, kt, 0:P],
                            rhs=XK[kt][:, c0 + h0:c0 + h0 + hw],
                            start=(kt == 0), stop=(kt == NKT - 1),
                        )
                for kt in range(NKT):
                    for (h0, hw) in _chunks(cw, 512):
                        nc.tensor.matmul(
                            ps1[:, h0:h0 + hw],
                            lhsT=WK[:, kt, P:DQ],
                            rhs=XK[kt][:, c0 + h0:c0 + h0 + hw],
                            start=(kt == 0), stop=(kt == NKT - 1),
                        )
                nc.vector.tensor_scalar_add(
                    KT0[:, c0:c0 + cw], ps0[:, 0:cw], BK0
                )
                nc.vector.tensor_scalar_add(
                    KT1D[0:DH, c0:c0 + cw], ps1[0:DH, 0:cw], BK1
                )
            nc.sync.dma_start(out=KT1D[DH:P, :], in_=KT1D[0:DH, :])

            # ---- Q projection (one 1024-wide column chunk) ----
            def q_proj(c0, cw):
                ps0 = ps.tile([P, 1024], F32, tag="sp", bufs=2, name=f"qps0_{c0}")
                ps1 = ps.tile([DH, 1024], F32, tag="ctx01", bufs=1, name=f"qps1_{c0}")
                for kt in range(NKT):
                    for (h0, hw) in _chunks(cw, 512):
                        nc.tensor.matmul(
                            ps0[:, h0:h0 + hw],
                            lhsT=WQ[:, kt, 0:P],
                            rhs=XQ[kt][:, c0 + h0:c0 + h0 + hw],
                            start=(kt == 0), stop=(kt == NKT - 1),
                        )
                for kt in range(NKT):
                    for (h0, hw) in _chunks(cw, 512):
                        nc.tensor.matmul(
                            ps1[:, h0:h0 + hw],
                            lhsT=WQ[:, kt, P:DQ],
                            rhs=XQ[kt][:, c0 + h0:c0 + h0 + hw],
                            start=(kt == 0), stop=(kt == NKT - 1),
                        )
                nc.vector.tensor_scalar_add(
                    QT0[:, c0:c0 + cw], ps0[:, 0:cw], BQ0
                )
                nc.vector.tensor_scalar_add(
                    QT1D[0:DH, c0:c0 + cw], ps1[0:DH, 0:cw], BQ1
                )
                nc.sync.dma_start(
                    out=QT1D[DH:P, c0:c0 + cw], in_=QT1D[0:DH, c0:c0 + cw]
                )

            # only the first 512 Q columns are projected up front; each later
            # 512-piece is emitted just before the attention chunk that needs
            # it, so its matmuls fill the previous chunk's PE gaps
            q_proj(0, 512)

            # ---- V projection ----
            for t in range(T):
                vps = ps.tile([P, DQ], F32, tag="opo", bufs=1, name=f"vps{t}")
                for kt in range(NKT):
                    nc.tensor.matmul(
                        vps,
                        lhsT=XV[kt][:, t * P:(t + 1) * P],
                        rhs=WV[:, kt, :],
                        start=(kt == 0), stop=(kt == NKT - 1),
                    )
                vview = VP[:, t, :].rearrange("p (h c) -> p h c", c=P)
                nc.vector.tensor_add(
                    vview[:, :, 0:DH],
                    vps.rearrange("p (h d) -> p h d", d=DH),
                    BV[:, :].rearrange("p (h d) -> p h d", d=DH),
                )
                nc.vector.tensor_scalar_mul(
                    vview[:, :, 0:DH], vview[:, :, 0:DH], VM[:, t:t + 1]
                )
                nc.vector.tensor_copy(
                    vview[:, :, DH:DH + 1],
                    VM[:, t:t + 1].to_broadcast([P, HPG, 1]),
                )

            # ---- attention + output projection, per query chunk ----
            def attn_norm(ctxp, col, dst, cw, uid):
                # denominator -> SBUF -> reciprocal -> partition-broadcast,
                # then scale ctx rows 0:64 straight out of PSUM
                dn = norm.tile([1, 512], F32, tag="dn", name=f"dn{uid}")
                nc.vector.tensor_copy(
                    dn[:, 0:cw], ctxp[DH:DH + 1, col:col + cw]
                )
                rc = norm.tile([1, 512], F32, tag="rc", name=f"rc{uid}")
                nc.vector.reciprocal_approx_fast(rc[:, 0:cw], dn[:, 0:cw])
                bc = norm.tile([DH, 512], F32, tag="bc", name=f"bc{uid}")
                nc.gpsimd.partition_broadcast(bc[:, 0:cw], rc[:, 0:cw])
                nc.vector.tensor_mul(
                    dst[:, 0:cw], ctxp[0:DH, col:col + cw], bc[:, 0:cw]
                )

            def h01_scores_exp(ci, c0, cw, t):
                sp = ps.tile([P, 1024], F32, tag="sp", bufs=2, name=f"sp{ci}_{t}")
                nc.tensor.matmul(
                    sp[:, 0:cw],
                    lhsT=KT0[0:DH, t * P:(t + 1) * P],
                    rhs=QT0[0:DH, c0:c0 + cw],
                    start=True, stop=True,
                )
                nc.tensor.matmul(
                    sp[:, 512:512 + cw],
                    lhsT=KT0[DH:P, t * P:(t + 1) * P],
                    rhs=QT0[DH:P, c0:c0 + cw],
                    start=True, stop=True,
                )
                es = espool.tile([P, 1024], BF, tag="es", name=f"es{ci}_{t}")
                nc.scalar.activation(
                    es, sp, AFT.Exp, bias=0.0, scale=1.0 / math.sqrt(DH),
                )
                return es

            # O-proj piece: one (m-tile, n-chunk) matmul pair + evict + store.
            # Pieces for chunk c are emitted one-per-t-slot inside chunk c+1's
            # loop so the PE never drains a big O-proj block while ACT starves.
            def opo_piece(m, n0, nw, CTX01, CTX2, mi, last, evict_dve):
                def emit():
                    po = ps.tile(
                        [P, 384], F32,
                        tag="sp" if last else "opo",
                        bufs=2 if last else 1,
                        name=f"po{m}_{n0}",
                    )
                    nc.tensor.matmul(
                        po[:, 0:nw],
                        lhsT=CTX01[:, mi * P:(mi + 1) * P],
                        rhs=WO0[:, n0:n0 + nw],
                        start=True, stop=False,
                    )
                    nc.tensor.matmul(
                        po[:, 0:nw],
                        lhsT=CTX2[:, mi * P:(mi + 1) * P],
                        rhs=WO2[:, n0:n0 + nw],
                        start=False, stop=True,
                    )
                    po_sb = osb.tile([P, 384], BF, tag="posb", name=f"posb{m}_{n0}")
                    if evict_dve:
                        nc.vector.tensor_copy(po_sb[:, 0:nw], po[:, 0:nw])
                    else:
                        nc.scalar.copy(po_sb[:, 0:nw], po[:, 0:nw])
                    nc.sync.dma_start(
                        out=out[m * P:(m + 1) * P, n0:n0 + nw], in_=po_sb[:, 0:nw]
                    )
                return emit

            pend = []
            pre = {}
            NACH = len(ACH)
            for ci, (c0, cw) in enumerate(ACH):
                if ci >= 1:
                    q_proj(ci * 512, 512)
                ctxp = ps.tile([P, 1024], F32, tag="ctx01", bufs=1, name=f"ctx{ci}")
                ctx2p = ps.tile([P, 512], F32, tag="ctx2", bufs=1, name=f"ctx2_{ci}")
                pieces = pend
                pend = []
                for t in range(T):
                    es = pre.pop((ci, t), None)
                    if es is None:
                        es = h01_scores_exp(ci, c0, cw, t)
                    nc.tensor.matmul(
                        ctxp[:, 0:cw],
                        lhsT=VP[:, t, 0:P],
                        rhs=es[:, 0:cw],
                        start=(t == 0), stop=(t == T - 1),
                    )
                    nc.tensor.matmul(
                        ctxp[:, 512:512 + cw],
                        lhsT=VP[:, t, P:2 * P],
                        rhs=es[:, 512:512 + cw],
                        start=(t == 0), stop=(t == T - 1),
                    )
                    if t >= 1 and pieces:
                        pieces.pop(0)()
                for f in pieces:
                    f()
                # h0/h1 norms: one wide copy/recip/broadcast for both heads
                CTX01 = ctxs.tile([P, 512], BF, tag="ctx01", name=f"CTX01_{ci}")
                CTX2 = ctxs.tile([DH, 512], BF, tag="ctx2", name=f"CTX2_{ci}")
                dn = norm.tile([1, 1024], F32, tag="dn01", name=f"dn01_{ci}")
                nc.vector.tensor_copy(dn, ctxp[DH:DH + 1, 0:1024])
                rc = norm.tile([1, 1024], F32, tag="rc01", name=f"rc01_{ci}")
                nc.vector.reciprocal_approx_fast(rc, dn)
                bc = norm.tile([DH, 1024], F32, tag="bc01", name=f"bc01_{ci}")
                nc.gpsimd.partition_broadcast(bc, rc)
                nc.vector.tensor_mul(
                    CTX01[0:DH, :], ctxp[0:DH, 0:512], bc[:, 0:512]
                )
                nc.vector.tensor_mul(
                    CTX01[DH:P, :], ctxp[0:DH, 512:1024], bc[:, 512:1024]
                )
                # prefetch the next chunk's first scores+exp so ACT stays fed
                # across the chunk boundary
                if ci + 1 < NACH:
                    nco = ACH[ci + 1][0]
                    for tp in range(0):
                        pre[(ci + 1, tp)] = h01_scores_exp(ci + 1, nco, 512, tp)
                # head 2: key-tile pairs in row groups 0:64 / 64:128
                t = 0
                while t < T:
                    ln = min(2, T - t)
                    sp = ps.tile([P, 1024], F32, tag="sp", bufs=2, name=f"sp2_{ci}_{t}")
                    nc.tensor.matmul(
                        sp[:, 0:cw],
                        lhsT=KT1D[0:DH, t * P:(t + 1) * P],
                        rhs=QT1D[0:DH, c0:c0 + cw],
                        start=True, stop=True,
                    )
                    if ln == 2:
                        nc.tensor.matmul(
                            sp[:, 512:512 + cw],
                            lhsT=KT1D[DH:P, (t + 1) * P:(t + 2) * P],
                            rhs=QT1D[DH:P, c0:c0 + cw],
                            start=True, stop=True,
                        )
                    es = espool.tile([P, 1024], BF, tag="es", name=f"es2_{ci}_{t}")
                    nc.scalar.activation(
                        es[:, 0:ln * 512], sp[:, 0:ln * 512],
                        AFT.Exp, bias=0.0, scale=1.0 / math.sqrt(DH),
                    )
                    for i in range(ln):
                        nc.tensor.matmul(
                            ctx2p[:, 0:cw],
                            lhsT=VP[:, t + i, 2 * P:3 * P],
                            rhs=es[:, i * 512:i * 512 + cw],
                            start=(t + i == 0), stop=(t + i == T - 1),
                        )
                    t += ln
                # h2 norm (sole reader of ctx2p; h01 slot already released)
                attn_norm(ctx2p, 0, CTX2, cw, f"{ci}_2")
                # queue this chunk's O-proj pieces; the last chunk emits its
                # own (on the freed scores slots so rounds pipeline)
                last = ci == NACH - 1
                for mi in range(cw // P):
                    m = c0 // P + mi
                    for ni, (n0, nw) in enumerate(NCH):
                        pend.append(opo_piece(
                            m, n0, nw, CTX01, CTX2, mi, last,
                            (mi + ni) % 2 == 0 or not last,
                        ))
                if last:
                    for f in pend:
                        f()
                    pend = []
    nc.compile()
    return nc


def _get_prog(KP):
    if KP not in _prog_cache:
        _prog_cache[KP] = _build_nc(KP)
    return _prog_cache[KP]


def _run(inputs, trace=False):
    import ml_dtypes
    from concourse.bass_utils import run_bass_kernel_spmd

    BF = ml_dtypes.bfloat16

    def _warr(w):
        # [768, DQ] -> [128, NKT*DQ], contiguous per partition
        a = w.reshape(NKT, P, DQ).transpose(1, 0, 2).reshape(P, NKT * DQ)
        return np.ascontiguousarray(a).astype(BF)

    query = np.asarray(inputs["query"], dtype=np.float32)
    key = np.asarray(inputs["key"], dtype=np.float32)
    value = np.asarray(inputs["value"], dtype=np.float32)
    mask = np.asarray(inputs["mask"])
    Wq = np.asarray(inputs["Wq"], dtype=np.float32)
    bq = np.asarray(inputs["bq"], dtype=np.float32)
    Wk = np.asarray(inputs["Wk"], dtype=np.float32)
    bk = np.asarray(inputs["bk"], dtype=np.float32)
    Wv = np.asarray(inputs["Wv"], dtype=np.float32)
    bv = np.asarray(inputs["bv"], dtype=np.float32)
    Wo = np.asarray(inputs["Wo"], dtype=np.float32)
    bo = np.asarray(inputs["bo"], dtype=np.float32)

    idx = [np.nonzero(mask[b, 0, 0] != 0)[0] for b in range(B)]
    keff = [len(i) for i in idx]
    KP = max(P, ((max(keff) + P - 1) // P) * P)
    T = KP // P

    nc = _get_prog(KP)

    per_batch = {}
    for b in range(B):
        xqT = np.ascontiguousarray(query[b].T).astype(BF)
        xkT = np.zeros((DM, KP), dtype=BF)
        xkT[:, :keff[b]] = key[b][idx[b]].T.astype(BF)
        xvT = np.zeros((DM, KP), dtype=BF)
        xvT[:, :keff[b]] = value[b][idx[b]].T.astype(BF)
        vmf = np.zeros((KP,), dtype=np.float32)
        vmf[:keff[b]] = 1.0
        vm2 = np.ascontiguousarray(vmf.reshape(T, P).T)  # [128, T]
        per_batch[b] = (xqT, xkT, xvT, vm2)

    in_maps = []
    for core in range(NCORES):
        b, g = core // G, core % G
        xqT, xkT, xvT, vm2 = per_batch[b]
        sl = slice(g * DQ, (g + 1) * DQ)
        in_maps.append({
            "xqT": xqT,
            "xkT": xkT,
            "xvT": xvT,
            "wq": _warr(Wq[:, sl]),
            "wk": _warr(Wk[:, sl]),
            "wv": _warr(Wv[:, sl]),
            "wo": np.ascontiguousarray(Wo[sl, :]).astype(BF),
            "bq": np.ascontiguousarray(bq[sl].reshape(DQ, 1)),
            "bk": np.ascontiguousarray(bk[sl].reshape(DQ, 1)),
            "bv": np.ascontiguousarray(bv[sl].reshape(1, DQ)),
            "vm": vm2,
        })

    res = run_bass_kernel_spmd(nc, in_maps, list(range(NCORES)), trace=trace)

    outp = np.zeros((B, S, DM), dtype=np.float32)
    for core in range(NCORES):
        outp[core // G] += np.asarray(res.results[core]["out"], dtype=np.float32)
    outp += bo.reshape(1, 1, DM)
    return outp, res


def kernel(**inputs) -> np.ndarray:
    out, _ = _run(inputs, trace=False)
    return out


if __name__ == "__main__":
    nc = _build_nc(1152)
    print("build OK")
